# revision 61
# baseline (speedup 1.0000x reference)
"""Causal self-attention on 8 Trainium2 NeuronCores.

Tensor-parallel by heads: each core owns 2 of the 16 heads end-to-end
(QKV projection -> causal attention -> row-sharded output projection),
and the 8 partial projection outputs (written bf16) are summed on the
host.

Layout notes (per core):
  - x is pre-transposed on host to xT [C, B*T] so the contraction dim C
    lands on SBUF partitions with no on-device transpose of x.
  - QKV weights are sharded column-wise and reordered to
    [qA qB | kA kB | vA vB] (A/B = the core's two heads), so the
    qkvT = W^T @ xT matmuls directly produce QT/KT/VT with head A on
    partitions 0-63 and head B on partitions 64-127.
  - V is PE-transposed back to [token, feat]; each head gets an extra
    ones column (VA/VB = [v | 1], 65 cols).
  - scoresT[kpos, qpos] softmax needs no max subtraction (inputs are
    well-scaled); causal mask applied to exp tiles via gpsimd
    affine_select (index predicate, no mask tensor).
  - attnV runs TOKEN-major: the exp tile block [128 keys, 128 q] is the
    PE stationary operand and [v | 1] streams, so psum accumulates
    y[token, feat] with the softmax denominator in a psum COLUMN per
    head.  The normalize is then an exact per-partition reciprocal
    ([128,2], ~0.2us) + two tensor_scalar multiplies -- no single-lane
    512-element reciprocal, no partition broadcast, no DRAM bounce.
    Normalized y blocks are PE-transposed back to feature-major for the
    projection (transposes interleave into the next q-tile's score loop
    so the PE never waits on the normalize chain).
  - proj: out_partial = y_shard @ Wp_shard; V-bias and b_proj are folded
    into a host-precomputed per-core row bias (attn rows sum to 1).
"""

import os
import numpy as np
from contextlib import ExitStack

import concourse.bass as bass
import concourse.mybir as mybir
import concourse.tile as tile
from concourse import bacc

B, T, C, H, D = 2, 2048, 1024, 16, 64
NCORES = 8
HPC = H // NCORES          # heads per core = 2
BT = B * T                 # 4096 tokens
P = 128
KO = C // P                # 8 contraction chunks of 128
MT = 512                   # qkv m-tile (tokens)
NMT_B = T // MT            # 4 m-tiles per batch
QTW = 512                  # q tile width
NQT = T // QTW             # 4
NKB = T // P               # 16 k-blocks per batch
NJ = QTW // P              # 4 q-subblocks of 128 per q tile
SCALE = 1.0 / np.sqrt(D)   # 0.125
F32 = mybir.dt.float32

# dtype for every tensor that feeds the PE. bf16 streams at 1 cycle/row
# (vs 4 for fp32) and supports the normal ldweights path + FWL.
BF16 = mybir.dt.bfloat16
_MM = os.environ.get("KERNEL_MMDT", "bf16")
MMDT = {"bf16": BF16, "f32": F32}[_MM]

LAST_RESULT = None  # BassKernelResults of the most recent run (for profiling)


def build_nc():
    nc = bacc.Bacc(target_bir_lowering=False)

    xT_d = nc.dram_tensor("xT", [C, BT], MMDT, kind="ExternalInput")
    w_d = nc.dram_tensor("w", [C, 3 * HPC * D], MMDT, kind="ExternalInput")
    bqk_d = nc.dram_tensor("bqk", [P, 2], F32, kind="ExternalInput")
    wp_d = nc.dram_tensor("wp", [P, C], MMDT, kind="ExternalInput")
    id_d = nc.dram_tensor("ident", [P, P], MMDT, kind="ExternalInput")
    out_d = nc.dram_tensor("out", [BT, C], MMDT, kind="ExternalOutput")

    DBG = os.environ.get("KERNEL_DBG", "0") == "1"
    if DBG:
        dbg_e_d = nc.dram_tensor("dbg_e", [2, P, 2 * QTW], F32, kind="ExternalOutput")
        dbg_yn_d = nc.dram_tensor("dbg_yn", [4, P, P], F32, kind="ExternalOutput")
        dbg_rt_d = nc.dram_tensor("dbg_rt", [4, P, 2], F32, kind="ExternalOutput")
        dbg_pq_d = nc.dram_tensor("dbg_pq", [4, P, 130], F32, kind="ExternalOutput")

    xT3 = xT_d.ap().rearrange("(ko p) m -> p ko m", p=P)   # [128, 8, 4096]
    w3 = w_d.ap().rearrange("(ko p) n -> p ko n", p=P)     # [128, 8, 384]
    out2 = out_d.ap()                                       # [4096, 1024]

    Exp = mybir.ActivationFunctionType.Exp

    with tile.TileContext(nc) as tc, ExitStack() as ctx:
        const = ctx.enter_context(tc.tile_pool(name="const", bufs=1))
        big = ctx.enter_context(tc.tile_pool(name="big", bufs=1))
        xpool = ctx.enter_context(tc.tile_pool(name="xpool", bufs=2))
        epool = ctx.enter_context(tc.tile_pool(name="epool", bufs=10))
        spool = ctx.enter_context(tc.tile_pool(name="spool", bufs=2))
        opool = ctx.enter_context(tc.tile_pool(name="opool", bufs=4))
        ypool = ctx.enter_context(tc.tile_pool(name="ypool", bufs=8))
        psum = ctx.enter_context(tc.tile_pool(name="psum", bufs=1, space="PSUM"))

        # ---------------- constants ----------------
        # small tiles first so the PE warmup chain (pid/pwp) unblocks early
        id_sb = const.tile([P, P], MMDT)
        nc.sync.dma_start(out=id_sb[:], in_=id_d.ap())
        bqk_sb0 = const.tile([P, 2], F32)
        nc.sync.dma_start(out=bqk_sb0[:], in_=bqk_d.ap())
        wp_sb = const.tile([P, C], MMDT)
        for nh in range(C // QTW):
            nc.sync.dma_start(out=wp_sb[:, nh * QTW:(nh + 1) * QTW],
                              in_=wp_d.ap()[:, nh * QTW:(nh + 1) * QTW])
        # interleave the w chunks with the first m-tile's x chunks so the
        # first qkv matmul (needs w[0] + xm0[0]) unblocks after ~0.25 MB of
        # DMA instead of after the full 1.8 MB.
        # ...and spread the issue across three DGE queues: a single queue
        # issues one DMA per ~600 ns, which was gating the first qkv chain.
        w_sb = const.tile([P, KO, 3 * HPC * D], MMDT)
        xm0 = xpool.tile([P, KO, MT], MMDT, tag="xm")
        for k in range(KO):
            nc.sync.dma_start(out=w_sb[:, k, :], in_=w3[:, k, :])
            nc.scalar.dma_start(out=xm0[:, k, :], in_=xT3[:, k, 0:MT])
        # Pre-consume DMA semaphores on the engines that will read these
        # tiles: same-engine deps need no sync wait, and several encodings
        # (TensorScalarPtr) only have ONE wait slot, so a (PE, DMA) wait
        # pair fails walrus codegen.
        bqk_sb = const.tile([P, 2], F32)
        nc.vector.tensor_copy(out=bqk_sb[:], in_=bqk_sb0[:])
        # Tiny warmup matmuls: consume the id/wp DMA semaphores on the PE
        # (walrus wait-slot workaround) at ~50 ns each instead of full-size
        # transposes/matmuls that serialized ~4 us of startup.
        pid = psum.tile([P, P], F32, tag="py", bufs=2)
        nc.tensor.matmul(pid[0:2, 0:2], id_sb[:, 0:2], id_sb[:, 0:2],
                         start=True, stop=True)
        pwp = psum.tile([P, QTW], F32, tag="py", bufs=2)
        nc.tensor.matmul(pwp[0:2, 0:2], wp_sb[:, 0:2], wp_sb[:, 0:2],
                         start=True, stop=True)
        nc.tensor.matmul(pwp[0:2, 0:2], wp_sb[:, QTW:QTW + 2],
                         wp_sb[:, QTW:QTW + 2], start=True, stop=True)

        QT_sb = big.tile([P, B, T], MMDT)   # rows: [qA feats | qB feats]
        KT_sb = big.tile([P, B, T], MMDT)
        VT_sb = big.tile([P, B, T], MMDT)   # bf16 so transposes stream 1cyc/col
        VA_sb = big.tile([P, B, NKB, 65], MMDT)   # [vA | ones]
        VB_sb = big.tile([P, B, NKB, 65], MMDT)   # [vB | ones]

        nc.vector.memset(VA_sb[:, :, :, 64:65], 1.0)
        nc.vector.memset(VB_sb[:, :, :, 64:65], 1.0)

        yts = {}
        # Deferred per-q-tile tail work (attnV flush, normalize, transposes):
        # drained two items per kb iteration of the NEXT q tile so the next
        # tile's score/exp pipeline starts without an ACT bubble.
        defer = []

        def emit_qkv_dma(b, mt):
            g = b * NMT_B + mt
            if b == 0 and mt == 0:
                return xm0
            xm = xpool.tile([P, KO, MT], MMDT, tag="xm")
            for k in range(KO):
                nc.sync.dma_start(out=xm[:, k, :],
                                  in_=xT3[:, k, g * MT:(g + 1) * MT])
            return xm

        def emit_qkv_nch(b, mt, nch, xm):
            # ------------- QKV projection (transposed outputs) -------------
            pq = psum.tile([P, MT], F32, tag="py", bufs=2)
            for k in range(KO):
                nc.tensor.matmul(
                    pq[:],
                    (w_sb[:, k, nch * P:(nch + 1) * P]),
                    (xm[:, k, :]),
                    start=(k == 0),
                    stop=(k == KO - 1),
                )
            dst = (QT_sb, KT_sb, VT_sb)[nch]
            dslice = dst[:, b, mt * MT:(mt + 1) * MT]
            if nch < 2:
                nc.vector.tensor_scalar_add(
                    out=dslice, in0=pq[:], scalar1=bqk_sb[:, nch:nch + 1]
                )
            else:
                nc.vector.tensor_copy(out=dslice, in_=pq[:])

        def emit_qkv_mt(b, mt):
            xm = emit_qkv_dma(b, mt)
            for nch in range(3):
                emit_qkv_nch(b, mt, nch, xm)

        def emit_vtrans(b, kbs):
            # ------------- V back-transpose to [token, feat] -------------
            for kb in kbs:
                pt = psum.tile([P, P], MMDT, tag="py", bufs=2)
                nc.tensor.transpose(pt[:], VT_sb[:, b, kb * P:(kb + 1) * P], id_sb[:])
                nc.vector.tensor_copy(out=VA_sb[:, b, kb, 0:64], in_=pt[:, 0:64])
                nc.vector.tensor_copy(out=VB_sb[:, b, kb, 0:64], in_=pt[:, 64:128])

        def emit_proj_sm(b, sm):
            # ------------- output projection (row-sharded partial) -------------
            # proj bias is added on the host after the partial sum; psum
            # evacuation alternates DVE/ACT, partial output is written bf16,
            # and each row-tile's DMA is split across 4 rings (a single-ring
            # 256 KB transfer was the proj-phase bottleneck).
            osb = opool.tile([P, C], MMDT, tag="osb")
            for nh in range(C // QTW):
                # rotate po across the attention pools (idle during proj)
                # for a 4-deep psum pipeline instead of 2.
                if (sm + nh) % 2 == 0:
                    po = psum.tile([P, QTW], F32, tag="py", bufs=2)
                else:
                    po_w = psum.tile([P, 2 * QTW], F32, tag="ps", bufs=2,
                                     name=f"po_w_{b}_{sm}_{nh}")
                    po = po_w[:, 0:QTW]
                yTq = yts[(b, sm // 4)]
                nc.tensor.matmul(
                    po[:],
                    (yTq[:, (sm % 4) * P:(sm % 4 + 1) * P]),
                    (wp_sb[:, nh * QTW:(nh + 1) * QTW]),
                    start=True, stop=True,
                )
                if (sm + nh) % 2 == 0:
                    nc.vector.tensor_copy(
                        out=osb[:, nh * QTW:(nh + 1) * QTW], in_=po[:])
                else:
                    nc.scalar.copy(
                        out=osb[:, nh * QTW:(nh + 1) * QTW], in_=po[:])
            r0 = b * T + sm * P
            if b == B - 1 and sm >= 12:
                # split the final row tiles across two rings so the last
                # DMA is not an exposed single-ring tail.
                nc.sync.dma_start(out=out2[r0:r0 + P, 0:512], in_=osb[:, 0:512])
                nc.scalar.dma_start(out=out2[r0:r0 + P, 512:C], in_=osb[:, 512:C])
            else:
                nc.sync.dma_start(out=out2[r0:r0 + P, :], in_=osb[:])

        def emit_attn_qt(b, qt, fill=None):
            # ------------- causal attention, one q tile -------------
            # `fill` is a list of zero-arg emitters (other-batch qkv/proj
            # work) drained into the PE stream while ACT computes exps.
            nkb = (qt + 1) * NJ
            # token-major y psum: [q-subblock j, 130] = [yA|denomA|yB|denomB]
            # padded to 256 so each slot is exactly one bank and no matmul
            # dst region crosses a psum bank boundary.
            pyq_lo = psum.tile([P, 2, 256], F32, tag="pyq", bufs=2)
            pyq_hi = psum.tile([P, 2, 256], F32, tag="pyq", bufs=2)
            pyqs = (pyq_lo, pyq_hi)
            yTq = ypool.tile([P, QTW], MMDT, tag="yT")
            yts[(b, qt)] = yTq

            def emit_attnv(kb, e, qoff, b=b, qt=qt, pyqs=pyqs):
                d = kb - qt * NJ
                e3v = e.rearrange("p (h q) -> p h q", h=2)
                for j in range(max(0, d), NJ):
                    pyqj = pyqs[j // 2][:, j % 2, :]
                    # start_tensor_calc clears has_written for the WHOLE
                    # bank, so only the very first matmul into each pyq
                    # bank may carry it; every other region relies on the
                    # per-element has_written bit (clear -> overwrite,
                    # set -> accumulate).
                    st = (kb == 0 and j % 2 == 0)
                    sp = (kb == qt * NJ + j)
                    qs = slice(j * P, (j + 1) * P)
                    nc.tensor.matmul(
                        pyqj[:, 0:65], (e3v[:, 0, qs]), (VA_sb[:, b, kb, :]),
                        start=st, stop=sp, skip_group_check=True,
                    )
                    nc.tensor.matmul(
                        pyqj[:, 65:130], (e3v[:, 1, qs]), (VB_sb[:, b, kb, :]),
                        start=False, stop=sp, skip_group_check=True,
                    )

            pend = []
            for kb in range(nkb):
                # diagonal blocks only touch q positions >= 128*d; narrow
                # all work (scores, exp, mask, attnV) to that window.
                d = kb - (qt * NJ)
                qoff = max(0, d) * P
                w = QTW - qoff
                q0 = qt * QTW + qoff
                # Drain dependency-free PE work (deferred tail, attnV of
                # kb-skew) BEFORE the scores matmul: scores stalls on the
                # ps slot (exp of kb-2), and the PE is in-order, so free
                # work queued behind a stall is wasted slack.
                for _ in range(4):
                    if defer:
                        defer.pop(0)()
                skew = int(os.environ.get("KERNEL_SKEW", "5"))
                if len(pend) >= skew:
                    emit_attnv(*pend.pop(0))
                ps = psum.tile([P, 2 * QTW], F32, tag="ps", bufs=2)
                nc.tensor.matmul(
                    ps[:, qoff:QTW],
                    (KT_sb[0:64, b, kb * P:(kb + 1) * P]),
                    (QT_sb[0:64, b, q0:q0 + w]),
                    start=True, stop=True, tile_position=(0, 0),
                )
                nc.tensor.matmul(
                    ps[:, QTW + qoff:2 * QTW],
                    (KT_sb[64:128, b, kb * P:(kb + 1) * P]),
                    (QT_sb[64:128, b, q0:q0 + w]),
                    start=True, stop=True, tile_position=(64, 0),
                )
                e = epool.tile([P, 2 * QTW], MMDT, tag="e", bufs=10)
                ps3 = ps.rearrange("p (h q) -> p h q", h=2)
                e3 = e.rearrange("p (h q) -> p h q", h=2)
                nc.scalar.activation(out=e3[:, :, qoff:], in_=ps3[:, :, qoff:],
                                     func=Exp, scale=SCALE)
                if d >= 0:
                    # within the window: keep exp[j, h, i'] where i' >= j
                    nc.gpsimd.affine_select(
                        out=e3[:, :, qoff:],
                        in_=e3[:, :, qoff:],
                        pattern=[[0, 2], [1, w]],
                        compare_op=mybir.AluOpType.is_ge,
                        fill=0.0,
                        base=0,
                        channel_multiplier=-1,
                    )
                if DBG and b == 0 and qt == 0 and kb < 2:
                    est = spool.tile([P, 2 * QTW], F32, tag="dbg_e")
                    nc.vector.memset(est[:], 0.0)
                    est3 = est.rearrange("p (h q) -> p h q", h=2)
                    nc.vector.tensor_copy(out=est3[:, :, qoff:],
                                          in_=e3[:, :, qoff:])
                    nc.sync.dma_start(out=dbg_e_d.ap()[kb], in_=est[:])
                if fill:
                    fill.pop(0)()
                pend.append((kb, e, qoff))
            # Defer the tail (remaining attnV groups, normalize, transposes)
            # into the next q-tile's kb loop.
            for item in pend:
                defer.append(lambda item=item: emit_attnv(*item))

            def emit_norm_pair(jp, pyqs=pyqs, yTq=yTq, b=b, qt=qt):
                # Normalize token-major: exact reciprocal of the psum
                # denominator columns, then per-partition scaled evacuation.
                # Both reciprocals read ACROSS the two j-regions of the bank
                # (the B-column AP covers the bank's final matmul write), so
                # every DVE read of this psum bank is ordered after all PE
                # writes to it -- a same-bank PE-write/DVE-read overlap is a
                # fatal hardware error that the address-level race tracking
                # does not see.
                tile_ = pyqs[jp]
                rtA = spool.tile([P, 2], F32, tag="rt2", bufs=4)
                rtB = spool.tile([P, 2], F32, tag="rt2b", bufs=4)
                nc.vector.reciprocal(
                    out=rtA.rearrange("p (a c) -> p a c", c=1),
                    in_=tile_[:, :, 64:65])
                nc.vector.reciprocal(
                    out=rtB.rearrange("p (a c) -> p a c", c=1),
                    in_=tile_[:, :, 129:130])
                for jj in range(2):
                    j = 2 * jp + jj
                    pyqj = tile_[:, jj, :]
                    yn = spool.tile([P, P], MMDT, tag="yn", bufs=8)
                    nc.vector.tensor_scalar_mul(
                        out=yn[:, 0:64], in0=pyqj[:, 0:64],
                        scalar1=rtA[:, jj:jj + 1])
                    nc.vector.tensor_scalar_mul(
                        out=yn[:, 64:128], in0=pyqj[:, 65:129],
                        scalar1=rtB[:, jj:jj + 1])
                    emit_norm_tail(j, yn, pyqj, rtA, rtB, yTq, b, qt)

            def emit_norm_tail(j, yn, pyqj, rtA, rtB, yTq, b, qt):

                if DBG and b == 0 and qt == 0:
                    ynst = spool.tile([P, P], F32, tag="dbg_yn")
                    nc.vector.tensor_copy(out=ynst[:], in_=yn[:])
                    nc.sync.dma_start(out=dbg_yn_d.ap()[j], in_=ynst[:])
                    nc.sync.dma_start(out=dbg_rt_d.ap()[j % 2],
                                      in_=rtA[:] if j % 2 == 0 else rtB[:])
                    pqst = spool.tile([P, 130], F32, tag="dbg_pq")
                    nc.vector.tensor_copy(out=pqst[:], in_=pyqj[:, 0:130])
                    nc.sync.dma_start(out=dbg_pq_d.ap()[j], in_=pqst[:])

                def emit_tpose(yn=yn, yTq=yTq, j=j):
                    pyt = psum.tile([P, P], MMDT, tag="py", bufs=2)
                    nc.tensor.transpose(pyt[:], yn[:], id_sb[:])
                    nc.vector.tensor_copy(
                        out=yTq[:, j * P:(j + 1) * P], in_=pyt[:])
                defer.append(emit_tpose)

            for jp in range(2):
                defer.append(lambda jp=jp: emit_norm_pair(jp))
            # any fill work not drained by the kb loop
            if fill:
                for f in fill:
                    f()
                fill.clear()

        # ---------------- schedule ----------------
        # Dense phase-ordered emission: contiguous matmul phases keep the
        # PE HAM clock warm (spreading them into attention gaps measured
        # strictly worse -- everything dropped to the cold 1.2 GHz clock).
        for b in range(B):
            for mt in range(NMT_B):
                emit_qkv_mt(b, mt)
            emit_vtrans(b, range(NKB))
            for qt in range(NQT):
                emit_attn_qt(b, qt)
            while defer:
                defer.pop(0)()
            for sm in range(T // P):
                emit_proj_sm(b, sm)

    nc.finalize()
    return nc


def prep_inputs(x, W_qkv, b_qkv, W_proj, b_proj):
    """Host-side sharding: returns list of 8 per-core input dicts."""
    import ml_dtypes
    mmnp = np.float32 if _MM == "f32" else ml_dtypes.bfloat16
    x = np.asarray(x, dtype=np.float32)
    W_qkv = np.asarray(W_qkv, dtype=np.float32)
    b_qkv = np.asarray(b_qkv, dtype=np.float32)
    W_proj = np.asarray(W_proj, dtype=np.float32)
    b_proj = np.asarray(b_proj, dtype=np.float32)

    xT = np.ascontiguousarray(x.reshape(BT, C).T).astype(mmnp)   # [C, BT]
    ident = np.eye(P, dtype=np.float32).astype(mmnp)

    in_maps = []
    for c in range(NCORES):
        hA, hB = HPC * c, HPC * c + 1
        cols = []
        for part in range(3):                               # q, k, v
            for h in (hA, hB):
                cols.append(W_qkv[:, part * C + h * D: part * C + (h + 1) * D])
        w = np.ascontiguousarray(np.concatenate(cols, axis=1)).astype(mmnp)  # [C, 384]

        bq = np.concatenate([b_qkv[hA * D:(hA + 1) * D], b_qkv[hB * D:(hB + 1) * D]])
        bk = np.concatenate([b_qkv[C + hA * D: C + (hA + 1) * D],
                             b_qkv[C + hB * D: C + (hB + 1) * D]])
        bqk = np.ascontiguousarray(np.stack([bq, bk], axis=1))  # [128, 2]

        wp = np.ascontiguousarray(W_proj[c * P:(c + 1) * P, :]).astype(mmnp)

        in_maps.append({
            "xT": xT,
            "w": w,
            "bqk": bqk,
            "wp": wp,
            "ident": ident,
        })
    return in_maps


_NC_CACHE = None


def kernel(x, W_qkv, b_qkv, W_proj, b_proj):
    global _NC_CACHE, LAST_RESULT
    from concourse.bass_utils import run_bass_kernel_spmd

    if _NC_CACHE is None:
        _NC_CACHE = build_nc()
    nc = _NC_CACHE

    in_maps = prep_inputs(x, W_qkv, b_qkv, W_proj, b_proj)
    trace = os.environ.get("KERNEL_TRACE", "0") == "1"
    res = run_bass_kernel_spmd(nc, in_maps, list(range(NCORES)), trace=trace)
    LAST_RESULT = res

    acc = np.zeros((BT, C), dtype=np.float64)
    for r in res.results:
        acc += r["out"].astype(np.float64)
    # attn rows sum to 1, so the V bias contributes b_v @ W_proj to every
    # token row; add it and the proj bias here (exact, part of unshard).
    W_proj = np.asarray(W_proj, dtype=np.float32)
    b_qkv = np.asarray(b_qkv, dtype=np.float32)
    b_proj = np.asarray(b_proj, dtype=np.float32)
    acc += (b_qkv[2 * C:].astype(np.float64) @ W_proj.astype(np.float64)
            + b_proj.astype(np.float64))
    return acc.astype(np.float32).reshape(B, T, C)


# revision 62
# speedup vs baseline: 1.0625x; 1.0625x over previous
"""Causal self-attention on 8 Trainium2 NeuronCores.

Tensor-parallel by heads: each core owns 2 of the 16 heads end-to-end
(QKV projection -> causal attention -> row-sharded output projection),
and the 8 partial projection outputs (written bf16) are summed on the
host.

Layout notes (per core):
  - x is pre-transposed on host to xT [C, B*T] so the contraction dim C
    lands on SBUF partitions with no on-device transpose of x.
  - QKV weights are sharded column-wise and reordered to
    [qA qB | kA kB | vA vB] (A/B = the core's two heads), so the
    qkvT = W^T @ xT matmuls directly produce QT/KT/VT with head A on
    partitions 0-63 and head B on partitions 64-127.
  - V is PE-transposed back to [token, feat]; each head gets an extra
    ones column (VA/VB = [v | 1], 65 cols).
  - scoresT[kpos, qpos] softmax needs no max subtraction (inputs are
    well-scaled); causal mask applied to exp tiles via gpsimd
    affine_select (index predicate, no mask tensor).
  - attnV runs TOKEN-major: the exp tile block [128 keys, 128 q] is the
    PE stationary operand and [v | 1] streams, so psum accumulates
    y[token, feat] with the softmax denominator in a psum COLUMN per
    head.  The normalize is then an exact per-partition reciprocal
    ([128,2], ~0.2us) + two tensor_scalar multiplies -- no single-lane
    512-element reciprocal, no partition broadcast, no DRAM bounce.
    Normalized y blocks are PE-transposed back to feature-major for the
    projection (transposes interleave into the next q-tile's score loop
    so the PE never waits on the normalize chain).
  - proj: out_partial = y_shard @ Wp_shard; V-bias and b_proj are folded
    into a host-precomputed per-core row bias (attn rows sum to 1).
"""

import os
import numpy as np
from contextlib import ExitStack

import concourse.bass as bass
import concourse.mybir as mybir
import concourse.tile as tile
from concourse import bacc

B, T, C, H, D = 2, 2048, 1024, 16, 64
NCORES = 8
HPC = H // NCORES          # heads per core = 2
BT = B * T                 # 4096 tokens
P = 128
KO = C // P                # 8 contraction chunks of 128
MT = 512                   # qkv m-tile (tokens)
NMT_B = T // MT            # 4 m-tiles per batch
QTW = 512                  # q tile width
NQT = T // QTW             # 4
NKB = T // P               # 16 k-blocks per batch
NJ = QTW // P              # 4 q-subblocks of 128 per q tile
SCALE = 1.0 / np.sqrt(D)   # 0.125
F32 = mybir.dt.float32

# dtype for every tensor that feeds the PE. bf16 streams at 1 cycle/row
# (vs 4 for fp32) and supports the normal ldweights path + FWL.
BF16 = mybir.dt.bfloat16
_MM = os.environ.get("KERNEL_MMDT", "bf16")
MMDT = {"bf16": BF16, "f32": F32}[_MM]

LAST_RESULT = None  # BassKernelResults of the most recent run (for profiling)


def build_nc():
    nc = bacc.Bacc(target_bir_lowering=False)

    xT_d = nc.dram_tensor("xT", [C, BT], MMDT, kind="ExternalInput")
    w_d = nc.dram_tensor("w", [C, 3 * HPC * D], MMDT, kind="ExternalInput")
    bqk_d = nc.dram_tensor("bqk", [P, 2], F32, kind="ExternalInput")
    wp_d = nc.dram_tensor("wp", [P, C], MMDT, kind="ExternalInput")
    id_d = nc.dram_tensor("ident", [P, P], MMDT, kind="ExternalInput")
    out_d = nc.dram_tensor("out", [BT, C], MMDT, kind="ExternalOutput")

    DBG = os.environ.get("KERNEL_DBG", "0") == "1"
    if DBG:
        dbg_e_d = nc.dram_tensor("dbg_e", [2, P, 2 * QTW], F32, kind="ExternalOutput")
        dbg_yn_d = nc.dram_tensor("dbg_yn", [4, P, P], F32, kind="ExternalOutput")
        dbg_rt_d = nc.dram_tensor("dbg_rt", [4, P, 2], F32, kind="ExternalOutput")
        dbg_pq_d = nc.dram_tensor("dbg_pq", [4, P, 130], F32, kind="ExternalOutput")

    xT3 = xT_d.ap().rearrange("(ko p) m -> p ko m", p=P)   # [128, 8, 4096]
    w3 = w_d.ap().rearrange("(ko p) n -> p ko n", p=P)     # [128, 8, 384]
    out2 = out_d.ap()                                       # [4096, 1024]

    Exp = mybir.ActivationFunctionType.Exp

    with tile.TileContext(nc) as tc, ExitStack() as ctx:
        const = ctx.enter_context(tc.tile_pool(name="const", bufs=1))
        big = ctx.enter_context(tc.tile_pool(name="big", bufs=1))
        xpool = ctx.enter_context(tc.tile_pool(name="xpool", bufs=2))
        epool = ctx.enter_context(tc.tile_pool(name="epool", bufs=8))
        spool = ctx.enter_context(tc.tile_pool(name="spool", bufs=2))
        opool = ctx.enter_context(tc.tile_pool(name="opool", bufs=4))
        ypool = ctx.enter_context(tc.tile_pool(name="ypool", bufs=8))
        psum = ctx.enter_context(tc.tile_pool(name="psum", bufs=1, space="PSUM"))

        # ---------------- constants ----------------
        # small tiles first so the PE warmup chain (pid/pwp) unblocks early
        id_sb = const.tile([P, P], MMDT)
        nc.sync.dma_start(out=id_sb[:], in_=id_d.ap())
        bqk_sb0 = const.tile([P, 2], F32)
        nc.sync.dma_start(out=bqk_sb0[:], in_=bqk_d.ap())
        wp_sb = const.tile([P, C], MMDT)
        for nh in range(C // QTW):
            nc.sync.dma_start(out=wp_sb[:, nh * QTW:(nh + 1) * QTW],
                              in_=wp_d.ap()[:, nh * QTW:(nh + 1) * QTW])
        # interleave the w chunks with the first m-tile's x chunks so the
        # first qkv matmul (needs w[0] + xm0[0]) unblocks after ~0.25 MB of
        # DMA instead of after the full 1.8 MB.
        # ...and spread the issue across three DGE queues: a single queue
        # issues one DMA per ~600 ns, which was gating the first qkv chain.
        w_sb = const.tile([P, KO, 3 * HPC * D], MMDT)
        xm0 = xpool.tile([P, KO, MT], MMDT, tag="xm")
        for k in range(KO):
            nc.sync.dma_start(out=w_sb[:, k, :], in_=w3[:, k, :])
            nc.scalar.dma_start(out=xm0[:, k, :], in_=xT3[:, k, 0:MT])
        # Pre-consume DMA semaphores on the engines that will read these
        # tiles: same-engine deps need no sync wait, and several encodings
        # (TensorScalarPtr) only have ONE wait slot, so a (PE, DMA) wait
        # pair fails walrus codegen.
        bqk_sb = const.tile([P, 2], F32)
        nc.vector.tensor_copy(out=bqk_sb[:], in_=bqk_sb0[:])
        # Tiny warmup matmuls: consume the id/wp DMA semaphores on the PE
        # (walrus wait-slot workaround) at ~50 ns each instead of full-size
        # transposes/matmuls that serialized ~4 us of startup.
        pid = psum.tile([P, P], F32, tag="py", bufs=2)
        nc.tensor.matmul(pid[0:2, 0:2], id_sb[:, 0:2], id_sb[:, 0:2],
                         start=True, stop=True)
        pwp = psum.tile([P, QTW], F32, tag="py", bufs=2)
        nc.tensor.matmul(pwp[0:2, 0:2], wp_sb[:, 0:2], wp_sb[:, 0:2],
                         start=True, stop=True)
        nc.tensor.matmul(pwp[0:2, 0:2], wp_sb[:, QTW:QTW + 2],
                         wp_sb[:, QTW:QTW + 2], start=True, stop=True)

        QT_sb = big.tile([P, B, T], MMDT)   # rows: [qA feats | qB feats]
        KT_sb = big.tile([P, B, T], MMDT)
        VT_sb = big.tile([P, B, T], MMDT)   # bf16 so transposes stream 1cyc/col
        VA_sb = big.tile([P, B, NKB, 65], MMDT)   # [vA | ones]
        VB_sb = big.tile([P, B, NKB, 65], MMDT)   # [vB | ones]

        nc.vector.memset(VA_sb[:, :, :, 64:65], 1.0)
        nc.vector.memset(VB_sb[:, :, :, 64:65], 1.0)

        yts = {}
        # Deferred per-q-tile tail work (attnV flush, normalize, transposes):
        # drained two items per kb iteration of the NEXT q tile so the next
        # tile's score/exp pipeline starts without an ACT bubble.
        defer = []

        def emit_qkv_dma(b, mt):
            g = b * NMT_B + mt
            if b == 0 and mt == 0:
                return xm0
            xm = xpool.tile([P, KO, MT], MMDT, tag="xm")
            for k in range(KO):
                nc.sync.dma_start(out=xm[:, k, :],
                                  in_=xT3[:, k, g * MT:(g + 1) * MT])
            return xm

        def emit_qkv_nch(b, mt, nch, xm):
            # ------------- QKV projection (transposed outputs) -------------
            pq = psum.tile([P, MT], F32, tag="py", bufs=2)
            for k in range(KO):
                nc.tensor.matmul(
                    pq[:],
                    (w_sb[:, k, nch * P:(nch + 1) * P]),
                    (xm[:, k, :]),
                    start=(k == 0),
                    stop=(k == KO - 1),
                )
            dst = (QT_sb, KT_sb, VT_sb)[nch]
            dslice = dst[:, b, mt * MT:(mt + 1) * MT]
            if nch < 2:
                nc.vector.tensor_scalar_add(
                    out=dslice, in0=pq[:], scalar1=bqk_sb[:, nch:nch + 1]
                )
            else:
                nc.vector.tensor_copy(out=dslice, in_=pq[:])

        def emit_qkv_mt(b, mt):
            xm = emit_qkv_dma(b, mt)
            for nch in range(3):
                emit_qkv_nch(b, mt, nch, xm)

        def emit_vtrans(b, kbs):
            # ------------- V back-transpose to [token, feat] -------------
            for kb in kbs:
                pt = psum.tile([P, P], MMDT, tag="py", bufs=2)
                nc.tensor.transpose(pt[:], VT_sb[:, b, kb * P:(kb + 1) * P], id_sb[:])
                nc.vector.tensor_copy(out=VA_sb[:, b, kb, 0:64], in_=pt[:, 0:64])
                nc.vector.tensor_copy(out=VB_sb[:, b, kb, 0:64], in_=pt[:, 64:128])

        def emit_proj_sm(b, sm):
            # ------------- output projection (row-sharded partial) -------------
            # proj bias is added on the host after the partial sum; psum
            # evacuation alternates DVE/ACT, partial output is written bf16,
            # and each row-tile's DMA is split across 4 rings (a single-ring
            # 256 KB transfer was the proj-phase bottleneck).
            osb = opool.tile([P, C], MMDT, tag="osb")
            for nh in range(C // QTW):
                # rotate po across the attention pools (idle during proj)
                # for a 4-deep psum pipeline instead of 2.
                if (sm + nh) % 2 == 0:
                    po = psum.tile([P, QTW], F32, tag="py", bufs=2)
                else:
                    po_w = psum.tile([P, 2 * QTW], F32, tag="ps", bufs=2,
                                     name=f"po_w_{b}_{sm}_{nh}")
                    po = po_w[:, 0:QTW]
                yTq = yts[(b, sm // 4)]
                nc.tensor.matmul(
                    po[:],
                    (yTq[:, (sm % 4) * P:(sm % 4 + 1) * P]),
                    (wp_sb[:, nh * QTW:(nh + 1) * QTW]),
                    start=True, stop=True,
                )
                if (sm + nh) % 2 == 0:
                    nc.vector.tensor_copy(
                        out=osb[:, nh * QTW:(nh + 1) * QTW], in_=po[:])
                else:
                    nc.scalar.copy(
                        out=osb[:, nh * QTW:(nh + 1) * QTW], in_=po[:])
            r0 = b * T + sm * P
            if b == B - 1 and sm >= 12:
                # split the final row tiles across two rings so the last
                # DMA is not an exposed single-ring tail.
                nc.sync.dma_start(out=out2[r0:r0 + P, 0:512], in_=osb[:, 0:512])
                nc.scalar.dma_start(out=out2[r0:r0 + P, 512:C], in_=osb[:, 512:C])
            else:
                nc.sync.dma_start(out=out2[r0:r0 + P, :], in_=osb[:])

        def emit_attn_qt(b, qt, fill=None):
            # ------------- causal attention, one q tile -------------
            # `fill` is a list of zero-arg emitters (other-batch qkv/proj
            # work) drained into the PE stream while ACT computes exps.
            nkb = (qt + 1) * NJ
            # token-major y psum: [q-subblock j, 130] = [yA|denomA|yB|denomB]
            # padded to 256 so each slot is exactly one bank and no matmul
            # dst region crosses a psum bank boundary.
            pyq_lo = psum.tile([P, 2, 256], F32, tag="pyq", bufs=2)
            pyq_hi = psum.tile([P, 2, 256], F32, tag="pyq", bufs=2)
            pyqs = (pyq_lo, pyq_hi)
            yTq = ypool.tile([P, QTW], MMDT, tag="yT")
            yts[(b, qt)] = yTq

            def emit_attnv(kb, e, qoff, b=b, qt=qt, pyqs=pyqs):
                d = kb - qt * NJ
                e3v = e.rearrange("p (h q) -> p h q", h=2)
                for j in range(max(0, d), NJ):
                    pyqj = pyqs[j // 2][:, j % 2, :]
                    # start_tensor_calc clears has_written for the WHOLE
                    # bank, so only the very first matmul into each pyq
                    # bank may carry it; every other region relies on the
                    # per-element has_written bit (clear -> overwrite,
                    # set -> accumulate).
                    st = (kb == 0 and j % 2 == 0)
                    sp = (kb == qt * NJ + j)
                    qs = slice(j * P, (j + 1) * P)
                    nc.tensor.matmul(
                        pyqj[:, 0:65], (e3v[:, 0, qs]), (VA_sb[:, b, kb, :]),
                        start=st, stop=sp, skip_group_check=True,
                    )
                    nc.tensor.matmul(
                        pyqj[:, 65:130], (e3v[:, 1, qs]), (VB_sb[:, b, kb, :]),
                        start=False, stop=sp, skip_group_check=True,
                    )

            pend = []
            for kb in range(nkb):
                # diagonal blocks only touch q positions >= 128*d; narrow
                # all work (scores, exp, mask, attnV) to that window.
                d = kb - (qt * NJ)
                qoff = max(0, d) * P
                w = QTW - qoff
                q0 = qt * QTW + qoff
                # Drain dependency-free PE work (deferred tail, attnV of
                # kb-skew) BEFORE the scores matmul: scores stalls on the
                # ps slot (exp of kb-2), and the PE is in-order, so free
                # work queued behind a stall is wasted slack.
                for _ in range(3):
                    if defer:
                        defer.pop(0)()
                skew = int(os.environ.get("KERNEL_SKEW", "5"))
                if len(pend) >= skew:
                    emit_attnv(*pend.pop(0))
                ps = psum.tile([P, 2 * QTW], F32, tag="ps", bufs=2)
                nc.tensor.matmul(
                    ps[:, qoff:QTW],
                    (KT_sb[0:64, b, kb * P:(kb + 1) * P]),
                    (QT_sb[0:64, b, q0:q0 + w]),
                    start=True, stop=True, tile_position=(0, 0),
                )
                nc.tensor.matmul(
                    ps[:, QTW + qoff:2 * QTW],
                    (KT_sb[64:128, b, kb * P:(kb + 1) * P]),
                    (QT_sb[64:128, b, q0:q0 + w]),
                    start=True, stop=True, tile_position=(64, 0),
                )
                e = epool.tile([P, 2 * QTW], MMDT, tag="e", bufs=8)
                ps3 = ps.rearrange("p (h q) -> p h q", h=2)
                e3 = e.rearrange("p (h q) -> p h q", h=2)
                nc.scalar.activation(out=e3[:, :, qoff:], in_=ps3[:, :, qoff:],
                                     func=Exp, scale=SCALE)
                if d >= 0:
                    # within the window: keep exp[j, h, i'] where i' >= j
                    nc.gpsimd.affine_select(
                        out=e3[:, :, qoff:],
                        in_=e3[:, :, qoff:],
                        pattern=[[0, 2], [1, w]],
                        compare_op=mybir.AluOpType.is_ge,
                        fill=0.0,
                        base=0,
                        channel_multiplier=-1,
                    )
                if DBG and b == 0 and qt == 0 and kb < 2:
                    est = spool.tile([P, 2 * QTW], F32, tag="dbg_e")
                    nc.vector.memset(est[:], 0.0)
                    est3 = est.rearrange("p (h q) -> p h q", h=2)
                    nc.vector.tensor_copy(out=est3[:, :, qoff:],
                                          in_=e3[:, :, qoff:])
                    nc.sync.dma_start(out=dbg_e_d.ap()[kb], in_=est[:])
                if fill:
                    fill.pop(0)()
                pend.append((kb, e, qoff))
            # Defer the tail (remaining attnV groups, normalize, transposes)
            # into the next q-tile's kb loop.
            for item in pend:
                defer.append(lambda item=item: emit_attnv(*item))

            def emit_norm_pair(jp, pyqs=pyqs, yTq=yTq, b=b, qt=qt):
                # Normalize token-major: exact reciprocal of the psum
                # denominator columns, then per-partition scaled evacuation.
                # Both reciprocals read ACROSS the two j-regions of the bank
                # (the B-column AP covers the bank's final matmul write), so
                # every DVE read of this psum bank is ordered after all PE
                # writes to it -- a same-bank PE-write/DVE-read overlap is a
                # fatal hardware error that the address-level race tracking
                # does not see.
                tile_ = pyqs[jp]
                rtA = spool.tile([P, 2], F32, tag="rt2", bufs=4)
                rtB = spool.tile([P, 2], F32, tag="rt2b", bufs=4)
                nc.vector.reciprocal(
                    out=rtA.rearrange("p (a c) -> p a c", c=1),
                    in_=tile_[:, :, 64:65])
                nc.vector.reciprocal(
                    out=rtB.rearrange("p (a c) -> p a c", c=1),
                    in_=tile_[:, :, 129:130])
                for jj in range(2):
                    j = 2 * jp + jj
                    pyqj = tile_[:, jj, :]
                    yn = spool.tile([P, P], MMDT, tag="yn", bufs=8)
                    nc.vector.tensor_scalar_mul(
                        out=yn[:, 0:64], in0=pyqj[:, 0:64],
                        scalar1=rtA[:, jj:jj + 1])
                    nc.vector.tensor_scalar_mul(
                        out=yn[:, 64:128], in0=pyqj[:, 65:129],
                        scalar1=rtB[:, jj:jj + 1])
                    emit_norm_tail(j, yn, pyqj, rtA, rtB, yTq, b, qt)

            def emit_norm_tail(j, yn, pyqj, rtA, rtB, yTq, b, qt):

                if DBG and b == 0 and qt == 0:
                    ynst = spool.tile([P, P], F32, tag="dbg_yn")
                    nc.vector.tensor_copy(out=ynst[:], in_=yn[:])
                    nc.sync.dma_start(out=dbg_yn_d.ap()[j], in_=ynst[:])
                    nc.sync.dma_start(out=dbg_rt_d.ap()[j % 2],
                                      in_=rtA[:] if j % 2 == 0 else rtB[:])
                    pqst = spool.tile([P, 130], F32, tag="dbg_pq")
                    nc.vector.tensor_copy(out=pqst[:], in_=pyqj[:, 0:130])
                    nc.sync.dma_start(out=dbg_pq_d.ap()[j], in_=pqst[:])

                def emit_tpose(yn=yn, yTq=yTq, j=j):
                    pyt = psum.tile([P, P], MMDT, tag="py", bufs=2)
                    nc.tensor.transpose(pyt[:], yn[:], id_sb[:])
                    nc.vector.tensor_copy(
                        out=yTq[:, j * P:(j + 1) * P], in_=pyt[:])
                defer.append(emit_tpose)

            for jp in range(2):
                defer.append(lambda jp=jp: emit_norm_pair(jp))
            # any fill work not drained by the kb loop
            if fill:
                for f in fill:
                    f()
                fill.clear()

        # ---------------- schedule ----------------
        # Dense phase-ordered emission: contiguous matmul phases keep the
        # PE HAM clock warm (spreading them into attention gaps measured
        # strictly worse -- everything dropped to the cold 1.2 GHz clock).
        for b in range(B):
            for mt in range(NMT_B):
                emit_qkv_mt(b, mt)
            emit_vtrans(b, range(NKB))
            for qt in range(NQT):
                emit_attn_qt(b, qt)
            while defer:
                defer.pop(0)()
            for sm in range(T // P):
                emit_proj_sm(b, sm)

    nc.finalize()
    return nc


def prep_inputs(x, W_qkv, b_qkv, W_proj, b_proj):
    """Host-side sharding: returns list of 8 per-core input dicts."""
    import ml_dtypes
    mmnp = np.float32 if _MM == "f32" else ml_dtypes.bfloat16
    x = np.asarray(x, dtype=np.float32)
    W_qkv = np.asarray(W_qkv, dtype=np.float32)
    b_qkv = np.asarray(b_qkv, dtype=np.float32)
    W_proj = np.asarray(W_proj, dtype=np.float32)
    b_proj = np.asarray(b_proj, dtype=np.float32)

    xT = np.ascontiguousarray(x.reshape(BT, C).T).astype(mmnp)   # [C, BT]
    ident = np.eye(P, dtype=np.float32).astype(mmnp)

    in_maps = []
    for c in range(NCORES):
        hA, hB = HPC * c, HPC * c + 1
        cols = []
        for part in range(3):                               # q, k, v
            for h in (hA, hB):
                cols.append(W_qkv[:, part * C + h * D: part * C + (h + 1) * D])
        w = np.ascontiguousarray(np.concatenate(cols, axis=1)).astype(mmnp)  # [C, 384]

        bq = np.concatenate([b_qkv[hA * D:(hA + 1) * D], b_qkv[hB * D:(hB + 1) * D]])
        bk = np.concatenate([b_qkv[C + hA * D: C + (hA + 1) * D],
                             b_qkv[C + hB * D: C + (hB + 1) * D]])
        bqk = np.ascontiguousarray(np.stack([bq, bk], axis=1))  # [128, 2]

        wp = np.ascontiguousarray(W_proj[c * P:(c + 1) * P, :]).astype(mmnp)

        in_maps.append({
            "xT": xT,
            "w": w,
            "bqk": bqk,
            "wp": wp,
            "ident": ident,
        })
    return in_maps


_NC_CACHE = None


def kernel(x, W_qkv, b_qkv, W_proj, b_proj):
    global _NC_CACHE, LAST_RESULT
    from concourse.bass_utils import run_bass_kernel_spmd

    if _NC_CACHE is None:
        _NC_CACHE = build_nc()
    nc = _NC_CACHE

    in_maps = prep_inputs(x, W_qkv, b_qkv, W_proj, b_proj)
    trace = os.environ.get("KERNEL_TRACE", "0") == "1"
    res = run_bass_kernel_spmd(nc, in_maps, list(range(NCORES)), trace=trace)
    LAST_RESULT = res

    acc = np.zeros((BT, C), dtype=np.float64)
    for r in res.results:
        acc += r["out"].astype(np.float64)
    # attn rows sum to 1, so the V bias contributes b_v @ W_proj to every
    # token row; add it and the proj bias here (exact, part of unshard).
    W_proj = np.asarray(W_proj, dtype=np.float32)
    b_qkv = np.asarray(b_qkv, dtype=np.float32)
    b_proj = np.asarray(b_proj, dtype=np.float32)
    acc += (b_qkv[2 * C:].astype(np.float64) @ W_proj.astype(np.float64)
            + b_proj.astype(np.float64))
    return acc.astype(np.float32).reshape(B, T, C)
